# revision 41
# baseline (speedup 1.0000x reference)
"""BuddyPool kernel for Trainium2 (Bass/Tile), 8-core data-parallel.

Problem: cue (64,5,1024), patches (64,32,32,1024) ->
  sim = einsum('bkd,bhwd->bkhw'); idx = argmax(sim over hw);
  roi = mean of boundary-clamped 3x3 patch window around idx  -> (64,5,1024)

Sharding: batch across 8 cores, 8 samples/core.

Per-core pipeline (sample-PAIR processing, 137.9us vs 152.1us baseline):
  - loads: 2 half-loads per sample on sync, NAT_BUFS=4 deep prefetch keeps
    the 360GB/s DMA bus saturated (patch stream alone is a 93us floor).
  - front: per-sample PE transposes (f32r ident, 1.5cyc/row) -> PSUM;
    PSUM->SBUF copies alternate DVE/Act (the copy rate gates the front);
    sim accumulates with zero-padded [128,10] lhsT tiles so both samples
    of a pair share one [10,HW] PSUM tile; sims lag transposes by LAG_D=2.
  - mid (per PAIR, split in two parts injected into the NEXT front's DVE
    stream so in-order queues never head-of-line block): top-8 max/
    max_index straight off sim PSUM; trimmed window math writes gather
    row-ids + valid/count weights interleaved into one [10,18] tile; ONE
    spread DMA -> [90,2]; one SWDGE gather of 90 window rows (f32);
    weight matrix on the Pool engine (keeps the DVE queue clean).
  - roi: [90,10]x[90,512] f32 matmuls into 1-bank PSUM tiles, Act copies
    out halves, one output DMA per pair. roi is issued AFTER the next
    front so it cannot stall that front's transposes at the PE queue head.
  - PSUM banks: 3 transpose bufs + 2 sim bufs ([10,1024]) + 1 roi = 8.
"""

import sys

if "/opt/trn_rl_repo" not in sys.path:
    sys.path.insert(0, "/opt/trn_rl_repo")

import numpy as np

import concourse.bass as bass
import concourse.tile as tile
from concourse import mybir
from concourse.masks import make_identity

P = 128
B = 64          # full batch
NCORES = 8
NS = B // NCORES  # samples per core
K = 5
K2 = 2 * K      # pair-rows
D = 1024
H = W = 32
HW = H * W      # 1024
NDC = D // P    # 8 d-chunks
NHWC = HW // P  # 8 hw-chunks
NPAIR = NS // 2
F32 = mybir.dt.float32
F32R = mybir.dt.float32r
U32 = mybir.dt.uint32

SIM_DT = F32R
TRANS_DT = F32R
IDENT_DT = F32R
LAG_D = 2
NAT_BUFS = 4
PT_BUFS = 10
PST_BUFS = 3
PS_BUFS = 3


def split_multiwait_ctrl(nc, max_waits=1):
    """Walrus (neuronxcc CoreV3) rejects instructions carrying more than
    one sync wait. Hoist excess waits onto same-engine NOPs emitted just
    before the instruction -- program order on the engine's sequencer makes
    this semantically identical (waits are a conjunction)."""
    n_split = 0
    for fn in nc.m.functions:
        for bb in fn.blocks:
            new_list = []
            for inst in bb.instructions:
                si = inst.sync_info
                lim = 1 if isinstance(inst, mybir.InstMatmult) else max_waits
                if si is not None and si.on_wait and len(si.on_wait) > lim:
                    waits = list(si.on_wait)
                    extra, keep = waits[:-lim], waits[-lim:]
                    for i, w in enumerate(extra):
                        d = mybir.InstNoOp(
                            name=f"{inst.name}-ws{i}",
                            engine=inst.engine,
                            ins=[],
                            outs=[],
                            sync_info=mybir.SyncInfo(on_wait=[w], on_update=[]),
                        )
                        nc.register_instruction(d)
                        new_list.append(d)
                    si.on_wait = keep
                    n_split += 1
                new_list.append(inst)
            bb.instructions[:] = new_list
    return n_split


def build_bass():
    nc = bass.Bass(
        trn_type="TRN2",
        target_bir_lowering=False,
        debug=False,
        enable_asserts=False,
    )

    cue_d = nc.dram_tensor("cue", [NS * K, D], F32, kind="ExternalInput").ap()
    pat_d = nc.dram_tensor("patches", [NS * HW, D], F32, kind="ExternalInput").ap()
    drt_d = nc.dram_tensor("drt", [K2, 9], F32, kind="ExternalInput").ap()
    dct_d = nc.dram_tensor("dct", [K2, 9], F32, kind="ExternalInput").ap()
    wsel_d = nc.dram_tensor("wsel", [K2 * 9, K2], F32, kind="ExternalInput").ap()
    sbase_d = nc.dram_tensor("sbase", [K2, NPAIR], F32, kind="ExternalInput").ap()
    out_d = nc.dram_tensor("out", [NS * K, D], F32, kind="ExternalOutput").ap()

    with tile.TileContext(nc) as tc:
        build_kernel(tc, out_d, cue_d, pat_d, drt_d, dct_d, wsel_d, sbase_d)
    split_multiwait_ctrl(nc, max_waits=1)
    return nc


def build_kernel(tc, out_d, cue_d, pat_d, drt_d, dct_d, wsel_d, sbase_d):
    nc = tc.nc
    from contextlib import ExitStack

    ctx = ExitStack()
    const = ctx.enter_context(tc.tile_pool(name="const", bufs=1))
    natp = ctx.enter_context(tc.tile_pool(name="nat", bufs=NAT_BUFS))
    ptp = ctx.enter_context(tc.tile_pool(name="pt", bufs=PT_BUFS))
    smallp = ctx.enter_context(tc.tile_pool(name="small", bufs=3))
    g45p = ctx.enter_context(tc.tile_pool(name="g45", bufs=2))
    scrp = ctx.enter_context(tc.tile_pool(name="scr", bufs=2))
    pst = ctx.enter_context(tc.tile_pool(name="ps_t", bufs=PST_BUFS, space="PSUM"))
    simp = ctx.enter_context(tc.tile_pool(name="ps_sim", bufs=2, space="PSUM"))
    roip = ctx.enter_context(tc.tile_pool(name="ps_roi", bufs=1, space="PSUM"))

    def load_nat(s):
        nat = natp.tile([P, NHWC, D], TRANS_DT, tag="nat")
        src = pat_d[s * HW : (s + 1) * HW, :].rearrange(
            "(c p) d -> p c d", p=P
        ).bitcast(TRANS_DT)
        nc.sync.dma_start(out=nat[:, : NHWC // 2], in_=src[:, : NHWC // 2])
        nc.sync.dma_start(out=nat[:, NHWC // 2 :], in_=src[:, NHWC // 2 :])
        return nat

    nat0 = load_nat(0)
    nat1 = load_nat(1)

    # ---- constants ----
    ident_f = const.tile([P, P], F32)
    make_identity(nc, ident_f[:])
    ident = const.tile([P, P], IDENT_DT)
    nc.vector.tensor_copy(out=ident[:], in_=ident_f[:])
    drt = const.tile([K2, 9], F32)
    dct = const.tile([K2, 9], F32)
    wsel = const.tile([K2 * 9, K2], F32)
    sbase = const.tile([K2, NPAIR], F32)
    nc.scalar.dma_start(out=drt[:], in_=drt_d[:])
    nc.scalar.dma_start(out=dct[:], in_=dct_d[:])
    nc.scalar.dma_start(out=wsel[:], in_=wsel_d[:])
    nc.scalar.dma_start(out=sbase[:], in_=sbase_d[:])

    # ---- cue -> cueT ----
    cue_sb = const.tile([NS * K, D], F32)
    nc.scalar.dma_start(out=cue_sb[:], in_=cue_d[:])
    ident_cue = const.tile([NS * K, NS * K], F32)
    make_identity(nc, ident_cue[:])
    # cueTz[:, dc, s, :]: cue_s columns in its pair-slot (s%2)*5..+5,
    # zeros in the other slot -- lets each pair-sim matmul write the full
    # [10, 512] PSUM tile (out base partition must be 0) while only its own
    # sample's rows accumulate nonzero values.
    cueTz = const.tile([P, NDC, NS, K2], SIM_DT)
    cueTz_f = const.tile([P, NDC, NS, K2], F32)
    nc.vector.memset(cueTz_f[:], 0.0)
    for dc in range(NDC):
        ps = pst.tile([P, 512], F32, tag="pst")
        nc.tensor.transpose(
            out=ps[:, : NS * K],
            in_=cue_sb[:, dc * P : (dc + 1) * P],
            identity=ident_cue[:],
        )
        for j in range(2):
            src_ap = ps[:, : NS * K].rearrange("p (s k) -> p s k", k=K)
            nc.vector.tensor_copy(
                out=cueTz_f[:, dc, j::2, j * K : (j + 1) * K],
                in_=src_ap[:, j::2, :],
            )
    nc.vector.tensor_copy(out=cueTz[:], in_=cueTz_f[:])

    # ---- per-PAIR pipeline ----
    def stage_front2(p, inject_a=None, inject_b=None):
        s0 = 2 * p
        nats = [nat0, nat1] if p == 0 else [load_nat(s0), load_nat(s0 + 1)]
        sim_ps = simp.tile([K2, HW], F32, tag="sim")

        # per sample j: sim rows j*5..j*5+5 of sim_ps via lhsT [128, 10]
        # where the other sample's 5 columns are ZERO -- instead, simply run
        # separate matmuls per sample into disjoint row ranges: PSUM rows
        # are per-partition, so rows 0-4 and 5-9 are independent targets.
        def sim_mms(j, dc, pt):
            for half in range(2):
                nc.tensor.matmul(
                    out=sim_ps[:, half * 512 : (half + 1) * 512],
                    lhsT=cueTz[:, dc, s0 + j, :],
                    rhs=pt[:, half * 512 : (half + 1) * 512],
                    start=(dc == 0 and j == 0),
                    stop=(dc == NDC - 1 and j == 1),
                    skip_group_check=True,
                )

        # samples strictly sequential: sample 1's ops would otherwise sit
        # at the in-order PE/DVE queue heads waiting for its loads, blocking
        # sample 0's (ready) work
        pending = []
        for j in range(2):
            for dc in range(NDC):
                pt = ptp.tile([P, HW], SIM_DT, tag="pt")
                for half in range(2):
                    ps = pst.tile([P, 512], TRANS_DT, tag="pst")
                    for q in range(4):
                        hwc = half * 4 + q
                        nc.tensor.matmul(
                            out=ps[:, q * P : (q + 1) * P],
                            lhsT=nats[j][:, hwc, dc * P : (dc + 1) * P],
                            rhs=ident[:],
                            is_transpose=True,
                            skip_group_check=True,
                        )
                    dst = pt[:, half * 512 : (half + 1) * 512]
                    if half == 0:
                        nc.vector.tensor_copy(out=dst, in_=ps[:])
                    else:
                        nc.scalar.copy(out=dst, in_=ps[:])
                pending.append((j, dc, pt))
                if len(pending) > LAG_D:
                    sim_mms(*pending.pop(0))
                # interleave the previous pair's mid chain into sample 0's
                # DVE stream so neither head-of-line blocks the other
                if j == 0 and dc == 1 and inject_a is not None:
                    inject_a()
                if j == 0 and dc == 3 and inject_b is not None:
                    inject_b()
        for item in pending:
            sim_mms(*item)
        return p, sim_ps

    def mid_a(p, sim_ps, st):
        # ---- argmax for both samples straight off the PSUM sim tile ----
        mx8 = smallp.tile([K2, 8], F32, tag="mx8")
        idx8 = smallp.tile([K2, 8], U32, tag="idx8")
        nc.vector.max(out=mx8[:], in_=sim_ps[:])
        nc.vector.max_index(out=idx8[:], in_max=mx8[:], in_values=sim_ps[:])
        st["idx8"] = idx8

    def mid_b(p, sim_ps, st):
        idx8 = st["idx8"]
        # ---- window math: row-ids + weights interleaved into gv [10,18] ----
        t9 = smallp.tile([K2, 9 * 6], F32, tag="t9")
        hh = t9[:, 0:9]
        ww = t9[:, 9:18]
        hc = t9[:, 18:27]
        wc = t9[:, 27:36]
        valid = t9[:, 36:45]
        gxf = t9[:, 45:54]
        cnt = smallp.tile([K2, 2], F32, tag="cnt")
        hw_u = smallp.tile([K2, 2], U32, tag="hwu")
        sc = smallp.tile([K2, 2], F32, tag="sc")
        gv = smallp.tile([K2, 18], U32, tag="gv")
        nc.vector.tensor_scalar(
            out=hw_u[:, 0:1], in0=idx8[:, 0:1], scalar1=5, scalar2=None,
            op0=mybir.AluOpType.logical_shift_right,
        )
        nc.vector.tensor_scalar(
            out=hw_u[:, 1:2], in0=idx8[:, 0:1], scalar1=31, scalar2=None,
            op0=mybir.AluOpType.bitwise_and,
        )
        nc.vector.tensor_copy(out=sc[:], in_=hw_u[:])  # uint32 -> f32
        nc.vector.tensor_scalar(
            out=hh, in0=drt[:], scalar1=sc[:, 0:1], scalar2=None,
            op0=mybir.AluOpType.add,
        )
        nc.vector.tensor_scalar(
            out=ww, in0=dct[:], scalar1=sc[:, 1:2], scalar2=None,
            op0=mybir.AluOpType.add,
        )
        nc.vector.tensor_scalar(
            out=hc, in0=hh, scalar1=0.0, scalar2=float(H - 1),
            op0=mybir.AluOpType.max, op1=mybir.AluOpType.min,
        )
        nc.vector.tensor_scalar(
            out=wc, in0=ww, scalar1=0.0, scalar2=float(W - 1),
            op0=mybir.AluOpType.max, op1=mybir.AluOpType.min,
        )
        nc.vector.tensor_tensor(out=hh, in0=hh, in1=hc, op=mybir.AluOpType.is_equal)
        nc.vector.tensor_tensor(out=ww, in0=ww, in1=wc, op=mybir.AluOpType.is_equal)
        nc.vector.tensor_tensor(
            out=valid, in0=hh, in1=ww, op=mybir.AluOpType.mult
        )
        nc.vector.tensor_scalar(
            out=valid, in0=valid, scalar1=0.0, scalar2=None,
            op0=mybir.AluOpType.add, op1=mybir.AluOpType.add,
            accum_out=cnt[:, 0:1],
        )
        nc.vector.reciprocal(out=cnt[:, 1:2], in_=cnt[:, 0:1])
        nc.vector.tensor_scalar(
            out=gv.bitcast(F32)[:, 1:18:2], in0=valid,
            scalar1=cnt[:, 1:2], scalar2=None, op0=mybir.AluOpType.mult,
        )
        # gather row ids hc*32 + wc + s*HW (s per pair-row via sbase column)
        nc.vector.tensor_scalar(
            out=gxf, in0=hc, scalar1=float(W), scalar2=sbase[:, p : p + 1],
            op0=mybir.AluOpType.mult, op1=mybir.AluOpType.add,
        )
        nc.vector.tensor_tensor(out=gxf, in0=gxf, in1=wc, op=mybir.AluOpType.add)
        nc.vector.tensor_copy(out=gv[:, 0:18:2], in_=gxf)  # f32 -> u32

        # ---- one spread DMA on the (otherwise idle) sync queue ----
        g2off = smallp.tile([K2 * 9, 2], U32, tag="g2off")
        nc.sync.dma_start(out=g2off[:], in_=gv[:])

        # ---- gather 90 window rows (f32) ----
        g45 = g45p.tile([K2 * 9, D], F32, tag="g45")
        nc.gpsimd.indirect_dma_start(
            out=g45[:],
            out_offset=None,
            in_=pat_d[:],
            in_offset=bass.IndirectOffsetOnAxis(ap=g2off[:, 0:1], axis=0),
        )
        w45 = smallp.tile([K2 * 9, K2], F32, tag="w45")
        # on Pool: a DVE op here would wait on the spread DMA at the DVE
        # queue head and serialize consecutive pairs' argmax chains
        nc.gpsimd.tensor_scalar(
            out=w45[:], in0=wsel[:], scalar1=g2off.bitcast(F32)[:, 1:2],
            scalar2=None, op0=mybir.AluOpType.mult,
        )
        st["roi_args"] = (p, w45, g45)

    def stage_roi(p, w45, g45):
        out_sb = scrp.tile([K2, D], F32, tag="outsb")
        for half in range(2):
            roi_ps = roip.tile([K2, 512], F32, tag="roi")
            nc.tensor.matmul(
                out=roi_ps[:],
                lhsT=w45[:],
                rhs=g45[:, half * 512 : (half + 1) * 512],
                start=True,
                stop=True,
                skip_group_check=True,
            )
            nc.scalar.copy(
                out=out_sb[:, half * 512 : (half + 1) * 512], in_=roi_ps[:]
            )
        nc.scalar.dma_start(
            out=out_d[2 * p * K : (2 * p + 2) * K, :], in_=out_sb[:]
        )

    pend_mid = None   # (p, sim_ps) awaiting its mid chain
    pend_roi = None   # st dict holding "roi_args" from the previous mid
    for p in range(NPAIR):
        st = {}
        if pend_mid is not None:
            pm, psim = pend_mid
            ia = lambda pm=pm, psim=psim, st=st: mid_a(pm, psim, st)
            ib = lambda pm=pm, psim=psim, st=st: mid_b(pm, psim, st)
        else:
            ia = ib = None
        fr = stage_front2(p, inject_a=ia, inject_b=ib)
        # roi AFTER the front: at the PE queue head it would stall every
        # transpose of this pair behind the (p-2) gather chain
        if pend_roi is not None:
            stage_roi(*pend_roi["roi_args"])
        pend_roi = st if pend_mid is not None else None
        pend_mid = fr
    st = {}
    mid_a(pend_mid[0], pend_mid[1], st)
    mid_b(pend_mid[0], pend_mid[1], st)
    if pend_roi is not None:
        stage_roi(*pend_roi["roi_args"])
    stage_roi(*st["roi_args"])

    ctx.close()


def make_const_inputs():
    r = np.arange(9)
    dr = (r // 3 - 1).astype(np.float32)
    dc = (r % 3 - 1).astype(np.float32)
    drt = np.tile(dr[None, :], (K2, 1))
    dct = np.tile(dc[None, :], (K2, 1))
    wsel = np.zeros((K2 * 9, K2), np.float32)
    for kk in range(K2):
        wsel[9 * kk : 9 * (kk + 1), kk] = 1.0
    # sbase[row, pair] = sample-of-row * HW  (row<5 -> sample 2p, else 2p+1)
    sbase = np.zeros((K2, NPAIR), np.float32)
    for pp in range(NPAIR):
        sbase[:K, pp] = 2 * pp * HW
        sbase[K:, pp] = (2 * pp + 1) * HW
    return drt, dct, wsel, sbase


def make_in_maps(cue, patches):
    cue = np.ascontiguousarray(np.asarray(cue, np.float32)).reshape(B, K, D)
    patches = np.ascontiguousarray(np.asarray(patches, np.float32)).reshape(
        B, HW, D
    )
    drt, dct, wsel, sbase = make_const_inputs()
    in_maps = []
    for c in range(NCORES):
        in_maps.append(
            {
                "cue": np.ascontiguousarray(
                    cue[c * NS : (c + 1) * NS].reshape(NS * K, D)
                ),
                "patches": np.ascontiguousarray(
                    patches[c * NS : (c + 1) * NS].reshape(NS * HW, D)
                ),
                "drt": drt,
                "dct": dct,
                "wsel": wsel,
                "sbase": sbase,
            }
        )
    return in_maps


_NC_CACHE = None


def get_nc():
    global _NC_CACHE
    if _NC_CACHE is None:
        _NC_CACHE = build_bass()
    return _NC_CACHE


def run(cue, patches, trace=False):
    from concourse.bass_utils import run_bass_kernel_spmd

    nc = get_nc()
    in_maps = make_in_maps(cue, patches)
    res = run_bass_kernel_spmd(
        nc, in_maps, core_ids=list(range(NCORES)), trace=trace
    )
    outs = [r["out"].reshape(NS, K, D) for r in res.results]
    full = np.concatenate(outs, axis=0)
    return full, res


def kernel(cue, patches):
    full, _ = run(cue, patches, trace=False)
    return full


# revision 42
# speedup vs baseline: 1.0001x; 1.0001x over previous
"""BuddyPool kernel for Trainium2 (Bass/Tile), 8-core data-parallel.

Problem: cue (64,5,1024), patches (64,32,32,1024) ->
  sim = einsum('bkd,bhwd->bkhw'); idx = argmax(sim over hw);
  roi = mean of boundary-clamped 3x3 patch window around idx  -> (64,5,1024)

Sharding: batch across 8 cores, 8 samples/core.

Per-core pipeline (sample-PAIR processing, 137.9us vs 152.1us baseline):
  - loads: 2 half-loads per sample on sync, NAT_BUFS=4 deep prefetch keeps
    the 360GB/s DMA bus saturated (patch stream alone is a 93us floor).
  - front: per-sample PE transposes (f32r ident, 1.5cyc/row) -> PSUM;
    PSUM->SBUF copies alternate DVE/Act (the copy rate gates the front);
    sim accumulates with zero-padded [128,10] lhsT tiles so both samples
    of a pair share one [10,HW] PSUM tile; sims lag transposes by LAG_D=2.
  - mid (per PAIR, split in two parts injected into the NEXT front's DVE
    stream so in-order queues never head-of-line block): top-8 max/
    max_index straight off sim PSUM; trimmed window math writes gather
    row-ids + valid/count weights interleaved into one [10,18] tile; ONE
    spread DMA -> [90,2]; one SWDGE gather of 90 window rows (f32);
    weight matrix on the Pool engine (keeps the DVE queue clean).
  - roi: [90,10]x[90,512] f32 matmuls into 1-bank PSUM tiles, Act copies
    out halves, one output DMA per pair. roi is issued AFTER the next
    front so it cannot stall that front's transposes at the PE queue head.
  - PSUM banks: 3 transpose bufs + 2 sim bufs ([10,1024]) + 1 roi = 8.
"""

import sys

if "/opt/trn_rl_repo" not in sys.path:
    sys.path.insert(0, "/opt/trn_rl_repo")

import numpy as np

import concourse.bass as bass
import concourse.tile as tile
from concourse import mybir
from concourse.masks import make_identity

P = 128
B = 64          # full batch
NCORES = 8
NS = B // NCORES  # samples per core
K = 5
K2 = 2 * K      # pair-rows
D = 1024
H = W = 32
HW = H * W      # 1024
NDC = D // P    # 8 d-chunks
NHWC = HW // P  # 8 hw-chunks
NPAIR = NS // 2
F32 = mybir.dt.float32
F32R = mybir.dt.float32r
U32 = mybir.dt.uint32

SIM_DT = F32R
TRANS_DT = F32R
IDENT_DT = F32R
LAG_D = 2
NAT_BUFS = 4
PT_BUFS = 12
PST_BUFS = 3
PS_BUFS = 3


def split_multiwait_ctrl(nc, max_waits=1):
    """Walrus (neuronxcc CoreV3) rejects instructions carrying more than
    one sync wait. Hoist excess waits onto same-engine NOPs emitted just
    before the instruction -- program order on the engine's sequencer makes
    this semantically identical (waits are a conjunction)."""
    n_split = 0
    for fn in nc.m.functions:
        for bb in fn.blocks:
            new_list = []
            for inst in bb.instructions:
                si = inst.sync_info
                lim = 1 if isinstance(inst, mybir.InstMatmult) else max_waits
                if si is not None and si.on_wait and len(si.on_wait) > lim:
                    waits = list(si.on_wait)
                    extra, keep = waits[:-lim], waits[-lim:]
                    for i, w in enumerate(extra):
                        d = mybir.InstNoOp(
                            name=f"{inst.name}-ws{i}",
                            engine=inst.engine,
                            ins=[],
                            outs=[],
                            sync_info=mybir.SyncInfo(on_wait=[w], on_update=[]),
                        )
                        nc.register_instruction(d)
                        new_list.append(d)
                    si.on_wait = keep
                    n_split += 1
                new_list.append(inst)
            bb.instructions[:] = new_list
    return n_split


def build_bass():
    nc = bass.Bass(
        trn_type="TRN2",
        target_bir_lowering=False,
        debug=False,
        enable_asserts=False,
    )

    cue_d = nc.dram_tensor("cue", [NS * K, D], F32, kind="ExternalInput").ap()
    pat_d = nc.dram_tensor("patches", [NS * HW, D], F32, kind="ExternalInput").ap()
    drt_d = nc.dram_tensor("drt", [K2, 9], F32, kind="ExternalInput").ap()
    dct_d = nc.dram_tensor("dct", [K2, 9], F32, kind="ExternalInput").ap()
    wsel_d = nc.dram_tensor("wsel", [K2 * 9, K2], F32, kind="ExternalInput").ap()
    sbase_d = nc.dram_tensor("sbase", [K2, NPAIR], F32, kind="ExternalInput").ap()
    out_d = nc.dram_tensor("out", [NS * K, D], F32, kind="ExternalOutput").ap()

    with tile.TileContext(nc) as tc:
        build_kernel(tc, out_d, cue_d, pat_d, drt_d, dct_d, wsel_d, sbase_d)
    split_multiwait_ctrl(nc, max_waits=1)
    return nc


def build_kernel(tc, out_d, cue_d, pat_d, drt_d, dct_d, wsel_d, sbase_d):
    nc = tc.nc
    from contextlib import ExitStack

    ctx = ExitStack()
    const = ctx.enter_context(tc.tile_pool(name="const", bufs=1))
    natp = ctx.enter_context(tc.tile_pool(name="nat", bufs=NAT_BUFS))
    ptp = ctx.enter_context(tc.tile_pool(name="pt", bufs=PT_BUFS))
    smallp = ctx.enter_context(tc.tile_pool(name="small", bufs=3))
    g45p = ctx.enter_context(tc.tile_pool(name="g45", bufs=2))
    scrp = ctx.enter_context(tc.tile_pool(name="scr", bufs=2))
    pst = ctx.enter_context(tc.tile_pool(name="ps_t", bufs=PST_BUFS, space="PSUM"))
    simp = ctx.enter_context(tc.tile_pool(name="ps_sim", bufs=2, space="PSUM"))
    roip = ctx.enter_context(tc.tile_pool(name="ps_roi", bufs=1, space="PSUM"))

    def load_nat(s):
        nat = natp.tile([P, NHWC, D], TRANS_DT, tag="nat")
        src = pat_d[s * HW : (s + 1) * HW, :].rearrange(
            "(c p) d -> p c d", p=P
        ).bitcast(TRANS_DT)
        nc.sync.dma_start(out=nat[:, : NHWC // 2], in_=src[:, : NHWC // 2])
        nc.sync.dma_start(out=nat[:, NHWC // 2 :], in_=src[:, NHWC // 2 :])
        return nat

    nat0 = load_nat(0)
    nat1 = load_nat(1)

    # ---- constants ----
    ident_f = const.tile([P, P], F32)
    make_identity(nc, ident_f[:])
    ident = const.tile([P, P], IDENT_DT)
    nc.vector.tensor_copy(out=ident[:], in_=ident_f[:])
    drt = const.tile([K2, 9], F32)
    dct = const.tile([K2, 9], F32)
    wsel = const.tile([K2 * 9, K2], F32)
    sbase = const.tile([K2, NPAIR], F32)
    nc.scalar.dma_start(out=drt[:], in_=drt_d[:])
    nc.scalar.dma_start(out=dct[:], in_=dct_d[:])
    nc.scalar.dma_start(out=wsel[:], in_=wsel_d[:])
    nc.scalar.dma_start(out=sbase[:], in_=sbase_d[:])

    # ---- cue -> cueT ----
    cue_sb = const.tile([NS * K, D], F32)
    nc.scalar.dma_start(out=cue_sb[:], in_=cue_d[:])
    ident_cue = const.tile([NS * K, NS * K], F32)
    make_identity(nc, ident_cue[:])
    # cueTz[:, dc, s, :]: cue_s columns in its pair-slot (s%2)*5..+5,
    # zeros in the other slot -- lets each pair-sim matmul write the full
    # [10, 512] PSUM tile (out base partition must be 0) while only its own
    # sample's rows accumulate nonzero values.
    cueTz = const.tile([P, NDC, NS, K2], SIM_DT)
    cueTz_f = const.tile([P, NDC, NS, K2], F32)
    nc.vector.memset(cueTz_f[:], 0.0)
    for dc in range(NDC):
        ps = pst.tile([P, 512], F32, tag="pst")
        nc.tensor.transpose(
            out=ps[:, : NS * K],
            in_=cue_sb[:, dc * P : (dc + 1) * P],
            identity=ident_cue[:],
        )
        for j in range(2):
            src_ap = ps[:, : NS * K].rearrange("p (s k) -> p s k", k=K)
            nc.vector.tensor_copy(
                out=cueTz_f[:, dc, j::2, j * K : (j + 1) * K],
                in_=src_ap[:, j::2, :],
            )
    nc.vector.tensor_copy(out=cueTz[:], in_=cueTz_f[:])

    # ---- per-PAIR pipeline ----
    def stage_front2(p, inject_a=None, inject_b=None):
        s0 = 2 * p
        nats = [nat0, nat1] if p == 0 else [load_nat(s0), load_nat(s0 + 1)]
        sim_ps = simp.tile([K2, HW], F32, tag="sim")

        # per sample j: sim rows j*5..j*5+5 of sim_ps via lhsT [128, 10]
        # where the other sample's 5 columns are ZERO -- instead, simply run
        # separate matmuls per sample into disjoint row ranges: PSUM rows
        # are per-partition, so rows 0-4 and 5-9 are independent targets.
        def sim_mms(j, dc, pt):
            for half in range(2):
                nc.tensor.matmul(
                    out=sim_ps[:, half * 512 : (half + 1) * 512],
                    lhsT=cueTz[:, dc, s0 + j, :],
                    rhs=pt[:, half * 512 : (half + 1) * 512],
                    start=(dc == 0 and j == 0),
                    stop=(dc == NDC - 1 and j == 1),
                    skip_group_check=True,
                )

        # samples strictly sequential: sample 1's ops would otherwise sit
        # at the in-order PE/DVE queue heads waiting for its loads, blocking
        # sample 0's (ready) work
        pending = []
        for j in range(2):
            for dc in range(NDC):
                pt = ptp.tile([P, HW], SIM_DT, tag="pt")
                for half in range(2):
                    ps = pst.tile([P, 512], TRANS_DT, tag="pst")
                    for q in range(4):
                        hwc = half * 4 + q
                        nc.tensor.matmul(
                            out=ps[:, q * P : (q + 1) * P],
                            lhsT=nats[j][:, hwc, dc * P : (dc + 1) * P],
                            rhs=ident[:],
                            is_transpose=True,
                            skip_group_check=True,
                        )
                    dst = pt[:, half * 512 : (half + 1) * 512]
                    if half == 0:
                        nc.vector.tensor_copy(out=dst, in_=ps[:])
                    else:
                        nc.scalar.copy(out=dst, in_=ps[:])
                pending.append((j, dc, pt))
                if len(pending) > LAG_D:
                    sim_mms(*pending.pop(0))
                # interleave the previous pair's mid chain into sample 0's
                # DVE stream so neither head-of-line blocks the other
                if j == 0 and dc == 1 and inject_a is not None:
                    inject_a()
                if j == 0 and dc == 3 and inject_b is not None:
                    inject_b()
        for item in pending:
            sim_mms(*item)
        return p, sim_ps

    def mid_a(p, sim_ps, st):
        # ---- argmax for both samples straight off the PSUM sim tile ----
        mx8 = smallp.tile([K2, 8], F32, tag="mx8")
        idx8 = smallp.tile([K2, 8], U32, tag="idx8")
        nc.vector.max(out=mx8[:], in_=sim_ps[:])
        nc.vector.max_index(out=idx8[:], in_max=mx8[:], in_values=sim_ps[:])
        st["idx8"] = idx8

    def mid_b(p, sim_ps, st):
        idx8 = st["idx8"]
        # ---- window math: row-ids + weights interleaved into gv [10,18] ----
        t9 = smallp.tile([K2, 9 * 6], F32, tag="t9")
        hh = t9[:, 0:9]
        ww = t9[:, 9:18]
        hc = t9[:, 18:27]
        wc = t9[:, 27:36]
        valid = t9[:, 36:45]
        gxf = t9[:, 45:54]
        cnt = smallp.tile([K2, 2], F32, tag="cnt")
        hw_u = smallp.tile([K2, 2], U32, tag="hwu")
        sc = smallp.tile([K2, 2], F32, tag="sc")
        gv = smallp.tile([K2, 18], U32, tag="gv")
        nc.vector.tensor_scalar(
            out=hw_u[:, 0:1], in0=idx8[:, 0:1], scalar1=5, scalar2=None,
            op0=mybir.AluOpType.logical_shift_right,
        )
        nc.vector.tensor_scalar(
            out=hw_u[:, 1:2], in0=idx8[:, 0:1], scalar1=31, scalar2=None,
            op0=mybir.AluOpType.bitwise_and,
        )
        nc.vector.tensor_copy(out=sc[:], in_=hw_u[:])  # uint32 -> f32
        nc.vector.tensor_scalar(
            out=hh, in0=drt[:], scalar1=sc[:, 0:1], scalar2=None,
            op0=mybir.AluOpType.add,
        )
        nc.vector.tensor_scalar(
            out=ww, in0=dct[:], scalar1=sc[:, 1:2], scalar2=None,
            op0=mybir.AluOpType.add,
        )
        nc.vector.tensor_scalar(
            out=hc, in0=hh, scalar1=0.0, scalar2=float(H - 1),
            op0=mybir.AluOpType.max, op1=mybir.AluOpType.min,
        )
        nc.vector.tensor_scalar(
            out=wc, in0=ww, scalar1=0.0, scalar2=float(W - 1),
            op0=mybir.AluOpType.max, op1=mybir.AluOpType.min,
        )
        nc.vector.tensor_tensor(out=hh, in0=hh, in1=hc, op=mybir.AluOpType.is_equal)
        nc.vector.tensor_tensor(out=ww, in0=ww, in1=wc, op=mybir.AluOpType.is_equal)
        nc.vector.tensor_tensor(
            out=valid, in0=hh, in1=ww, op=mybir.AluOpType.mult
        )
        nc.vector.tensor_scalar(
            out=valid, in0=valid, scalar1=0.0, scalar2=None,
            op0=mybir.AluOpType.add, op1=mybir.AluOpType.add,
            accum_out=cnt[:, 0:1],
        )
        nc.vector.reciprocal(out=cnt[:, 1:2], in_=cnt[:, 0:1])
        nc.vector.tensor_scalar(
            out=gv.bitcast(F32)[:, 1:18:2], in0=valid,
            scalar1=cnt[:, 1:2], scalar2=None, op0=mybir.AluOpType.mult,
        )
        # gather row ids hc*32 + wc + s*HW (s per pair-row via sbase column)
        nc.vector.tensor_scalar(
            out=gxf, in0=hc, scalar1=float(W), scalar2=sbase[:, p : p + 1],
            op0=mybir.AluOpType.mult, op1=mybir.AluOpType.add,
        )
        nc.vector.tensor_tensor(out=gxf, in0=gxf, in1=wc, op=mybir.AluOpType.add)
        nc.vector.tensor_copy(out=gv[:, 0:18:2], in_=gxf)  # f32 -> u32

        # ---- one spread DMA on the (otherwise idle) sync queue ----
        g2off = smallp.tile([K2 * 9, 2], U32, tag="g2off")
        nc.sync.dma_start(out=g2off[:], in_=gv[:])

        # ---- gather 90 window rows (f32) ----
        g45 = g45p.tile([K2 * 9, D], F32, tag="g45")
        nc.gpsimd.indirect_dma_start(
            out=g45[:],
            out_offset=None,
            in_=pat_d[:],
            in_offset=bass.IndirectOffsetOnAxis(ap=g2off[:, 0:1], axis=0),
        )
        w45 = smallp.tile([K2 * 9, K2], F32, tag="w45")
        # on Pool: a DVE op here would wait on the spread DMA at the DVE
        # queue head and serialize consecutive pairs' argmax chains
        nc.gpsimd.tensor_scalar(
            out=w45[:], in0=wsel[:], scalar1=g2off.bitcast(F32)[:, 1:2],
            scalar2=None, op0=mybir.AluOpType.mult,
        )
        st["roi_args"] = (p, w45, g45)

    def stage_roi(p, w45, g45):
        out_sb = scrp.tile([K2, D], F32, tag="outsb")
        for half in range(2):
            roi_ps = roip.tile([K2, 512], F32, tag="roi")
            nc.tensor.matmul(
                out=roi_ps[:],
                lhsT=w45[:],
                rhs=g45[:, half * 512 : (half + 1) * 512],
                start=True,
                stop=True,
                skip_group_check=True,
            )
            nc.scalar.copy(
                out=out_sb[:, half * 512 : (half + 1) * 512], in_=roi_ps[:]
            )
        nc.scalar.dma_start(
            out=out_d[2 * p * K : (2 * p + 2) * K, :], in_=out_sb[:]
        )

    pend_mid = None   # (p, sim_ps) awaiting its mid chain
    pend_roi = None   # st dict holding "roi_args" from the previous mid
    for p in range(NPAIR):
        st = {}
        if pend_mid is not None:
            pm, psim = pend_mid
            ia = lambda pm=pm, psim=psim, st=st: mid_a(pm, psim, st)
            ib = lambda pm=pm, psim=psim, st=st: mid_b(pm, psim, st)
        else:
            ia = ib = None
        fr = stage_front2(p, inject_a=ia, inject_b=ib)
        # roi AFTER the front: at the PE queue head it would stall every
        # transpose of this pair behind the (p-2) gather chain
        if pend_roi is not None:
            stage_roi(*pend_roi["roi_args"])
        pend_roi = st if pend_mid is not None else None
        pend_mid = fr
    st = {}
    mid_a(pend_mid[0], pend_mid[1], st)
    mid_b(pend_mid[0], pend_mid[1], st)
    if pend_roi is not None:
        stage_roi(*pend_roi["roi_args"])
    stage_roi(*st["roi_args"])

    ctx.close()


def make_const_inputs():
    r = np.arange(9)
    dr = (r // 3 - 1).astype(np.float32)
    dc = (r % 3 - 1).astype(np.float32)
    drt = np.tile(dr[None, :], (K2, 1))
    dct = np.tile(dc[None, :], (K2, 1))
    wsel = np.zeros((K2 * 9, K2), np.float32)
    for kk in range(K2):
        wsel[9 * kk : 9 * (kk + 1), kk] = 1.0
    # sbase[row, pair] = sample-of-row * HW  (row<5 -> sample 2p, else 2p+1)
    sbase = np.zeros((K2, NPAIR), np.float32)
    for pp in range(NPAIR):
        sbase[:K, pp] = 2 * pp * HW
        sbase[K:, pp] = (2 * pp + 1) * HW
    return drt, dct, wsel, sbase


def make_in_maps(cue, patches):
    cue = np.ascontiguousarray(np.asarray(cue, np.float32)).reshape(B, K, D)
    patches = np.ascontiguousarray(np.asarray(patches, np.float32)).reshape(
        B, HW, D
    )
    drt, dct, wsel, sbase = make_const_inputs()
    in_maps = []
    for c in range(NCORES):
        in_maps.append(
            {
                "cue": np.ascontiguousarray(
                    cue[c * NS : (c + 1) * NS].reshape(NS * K, D)
                ),
                "patches": np.ascontiguousarray(
                    patches[c * NS : (c + 1) * NS].reshape(NS * HW, D)
                ),
                "drt": drt,
                "dct": dct,
                "wsel": wsel,
                "sbase": sbase,
            }
        )
    return in_maps


_NC_CACHE = None


def get_nc():
    global _NC_CACHE
    if _NC_CACHE is None:
        _NC_CACHE = build_bass()
    return _NC_CACHE


def run(cue, patches, trace=False):
    from concourse.bass_utils import run_bass_kernel_spmd

    nc = get_nc()
    in_maps = make_in_maps(cue, patches)
    res = run_bass_kernel_spmd(
        nc, in_maps, core_ids=list(range(NCORES)), trace=trace
    )
    outs = [r["out"].reshape(NS, K, D) for r in res.results]
    full = np.concatenate(outs, axis=0)
    return full, res


def kernel(cue, patches):
    full, _ = run(cue, patches, trace=False)
    return full
